# revision 25
# baseline (speedup 1.0000x reference)
"""LSH cosine-of-Hamming retrieval kernel for 8 trn2 NeuronCores.

Math: reference computes cos((pi/d) * hamming(u, v)) for binary LSH codes
u = (emb1 @ r.T > 0), v = (emb2 @ r.T > 0), d = 1024 bits.
With +/-1 sign codes s_u = 2u-1, s_v = 2v-1:
    hamming = (d - s_u . s_v) / 2
    cos((pi/d) * hamming) = sin((pi/2d) * s_u.s_v)

Projection runs as a single fp16 matmul pass (fp16 runs at bf16 rate on
the PE; the measured sign-flip rate ~9e-5 contributes ~1.3e-2 rel err,
inside the 2e-2 budget). Binarization of the projection PSUMs is split
across two engines: the DVE emits +/-0.5 codes (tensor_scalar is_gt/sub)
and the Activation engine emits +/-1 codes (Sign). Per 128-bit chunk the
two sides always use complementary conventions, so every bit contributes
(+/-1)*(+/-0.5) and the code dot is uniformly s_u.s_v/2 -> Sin scale
pi/1024. (GpSimd cannot help: it has no PSUM port.)

Main code matmul is fp8 DoubleRow (integer-exact in PSUM f32), split into
two column-half passes so the first half only needs v-chunks 0-1; the
u-side projection chunks 1-3 are interleaved between main row-block
groups, keeping the PE saturated end to end. Output is written bf16
(halves output DMA) and upcast on the host.

Sharding (2x4 grid over 8 cores): core k computes the [2048, 2048] output
block for emb1 rows [(k//4)*2048...] x emb2 rows [(k%4)*2048...]; r is
replicated (collectives cost ~60us fixed here - more than the projection
work they would dedupe).
"""

import sys

sys.path.insert(0, "/opt/trn_rl_repo")

import ml_dtypes
import numpy as np

import concourse.bacc as bacc
import concourse.tile as tile
from concourse import mybir
from concourse.bass_utils import run_bass_kernel_spmd

N1, N2, D, B = 4096, 8192, 128, 1024  # emb1 rows, emb2 rows, dim, num_bits
G1, G2 = 2, 4
M1, M2 = N1 // G1, N2 // G2  # 2048 x 2048 output block per core
KC = B // 128  # 8 bit-chunks of 128
RW = 512  # projection row-chunk width
NW = 512  # main matmul psum half-width

# bit-chunk PAIRS (of the KC//2 psum groups) whose u-side codes are made by
# the Activation engine as +/-1 (Sign); their v-side partners go to the DVE
# as +/-0.5. The complement set is the reverse. Keeps the convention product
# at 0.5 for every bit while splitting binarize work evenly per chunk.
SIGN_U_PAIRS = frozenset({0, 1})

_BUILD_CACHE = {}


def _dedupe_ldweights(nc):
    """Drop back-to-back InstLdweights with identical operands on the PE
    queue. The pipeline emits one weight load per matmul; when consecutive
    matmuls share a stationary operand, the reload is pure overhead. Only
    loads carrying no semaphore waits/updates are removed, so sync
    arithmetic is unchanged; the paired matmuls then use the weights the
    earlier identical load left in the array."""
    removed = 0
    for f in nc.m.functions:
        for bb in f.blocks:
            last_key = None
            for ins in list(bb.instructions):
                if type(ins).__name__ == "InstLdweights":
                    key = ins.concise()
                    if (
                        key == last_key
                        and not ins.has_wait()
                        and not ins.has_update()
                    ):
                        bb.instructions.remove(ins)
                        removed += 1
                    else:
                        last_key = key
    return removed


def _build(scale: float):
    if scale in _BUILD_CACHE:
        return _BUILD_CACHE[scale]
    nc = bacc.Bacc("TRN2", target_bir_lowering=False, debug=False)
    f32 = mybir.dt.float32
    f16 = mybir.dt.float16
    bf16 = mybir.dt.bfloat16
    fp8 = mybir.dt.float8e4

    e1 = nc.declare_dram_parameter("e1", [D, M1], f16, isOutput=False)
    e2 = nc.declare_dram_parameter("e2", [D, M2], f16, isOutput=False)
    r = nc.declare_dram_parameter("r", [D, B], f16, isOutput=False)
    out = nc.declare_dram_parameter("out", [M1, M2], bf16, isOutput=True)

    with tile.TileContext(nc) as tc:
        with (
            tc.tile_pool(name="const", bufs=1) as const_pool,
            tc.tile_pool(name="codes", bufs=1) as code_pool,
            tc.tile_pool(name="outs", bufs=3) as out_pool,
            tc.tile_pool(name="psum", bufs=2, space="PSUM") as psum_pool,
            tc.tile_pool(name="pjp", bufs=2, space="PSUM") as ins_pool,
        ):
            r_sb = const_pool.tile([D, B], f16)
            e1_sb = const_pool.tile([D, M1], f16)
            e2_sb = const_pool.tile([D, M2], f16)
            ut = code_pool.tile([128, KC, M1], fp8)
            vt = code_pool.tile([128, KC, M2], fp8)

            # Input DMAs up front, in consumption order. sync queue carries
            # the gating pieces (first r bit-chunk, first e1 row-chunk);
            # gpsimd carries the e2 chunks. Output DMAs come later on both.
            # The DMA fabric drains each queue roughly in order at ~1.5us
            # latency + shared bandwidth, so the ~1.3MB input set is staged:
            # the prologue-gating transfers (r pieces in u0's bit-pair
            # order, e1 chunk 0, e2 chunk 0) lead the two queues, and the
            # four chunks not needed until the main phase ride the tails.
            def edma(q, esb, edr, j):
                sl = slice(j * RW, (j + 1) * RW)
                q.dma_start(esb[:, sl], edr[:, sl])

            nc.gpsimd.dma_start(e1_sb[:, 0:RW], e1[:, 0:RW])
            nc.sync.dma_start(r_sb[:, 512:768], r[:, 512:768])
            edma(nc.gpsimd, e2_sb, e2, 0)
            nc.sync.dma_start(r_sb[:, 0:512], r[:, 0:512])
            nc.sync.dma_start(r_sb[:, 768:], r[:, 768:])
            edma(nc.sync, e2_sb, e2, 1)
            edma(nc.sync, e1_sb, e1, 1)
            edma(nc.gpsimd, e2_sb, e2, 2)
            edma(nc.gpsimd, e2_sb, e2, 3)
            edma(nc.gpsimd, e1_sb, e1, 2)
            edma(nc.sync, e1_sb, e1, 3)

            # Narrow HAM warm-up matmuls bridge the input-DMA wait: an idle
            # gap before the first projection matmul would reset the PE
            # clock ramp (~3.4us of SUSTAINED activity needed for 2.4 GHz),
            # and 256-wide dummies waste at most ~0.2us once data lands.
            # The dummy Sin (into a scratch tile, so the warm-ups don't
            # depend on it) makes the activation table pass load the trig
            # table - which serves BOTH Sin and Sign - once, instead of
            # reloading at the first mid-stream function switch.
            warm = const_pool.tile([128, RW], bf16)
            scratch = const_pool.tile([128, 1], bf16)
            nc.vector.memset(warm[:], 0.0)
            nc.scalar.activation(
                scratch[:], warm[:, 0:1],
                mybir.ActivationFunctionType.Sin, scale=scale,
            )
            wps = psum_pool.tile([128, 2, RW], f32, name="pstile", tag="ps")
            for w in range(14):
                nc.tensor.matmul(
                    wps[:, w % 2, 0:256], warm[:, 0:128], warm[:, 0:256],
                    start=True, stop=True,
                )

            def proj_tile(side, j, c2, pool=psum_pool):
                esb = e1_sb if side == "u" else e2_sb
                dst = ut if side == "u" else vt
                sl = slice(j * RW, (j + 1) * RW)
                tg = "ps" if pool is psum_pool else "pj"
                ps = pool.tile([128, 2, RW], f32, name="pstile", tag=tg)
                for h in range(2):
                    cs = slice((2 * c2 + h) * 128, (2 * c2 + h + 1) * 128)
                    nc.tensor.matmul(
                        ps[:, h, :], r_sb[:, cs], esb[:, sl],
                        start=True, stop=True,
                    )
                code = dst[:, 2 * c2 : 2 * c2 + 2, sl]
                use_sign = (c2 in SIGN_U_PAIRS) == (side == "u")
                if use_sign:
                    nc.scalar.activation(
                        code, ps[:], mybir.ActivationFunctionType.Sign
                    )
                else:
                    nc.vector.tensor_scalar(
                        code,
                        ps[:],
                        0.0,
                        0.5,
                        mybir.AluOpType.is_gt,
                        mybir.AluOpType.subtract,
                    )

            # Strict prologue: only the tiles the first main block needs
            # (u rows 0-511, v columns 0-1023), each chunk's bit-pairs
            # alternating between the two binarize engines so neither
            # starves, and the tiles alternating between the two PSUM pools
            # so the effective rotation is 4 deep (a 2-deep rotation makes
            # the PE wait a binarize + two semaphore hops per tile). The u1
            # pre-inserts fill the tensor idle at the end of the gate (the
            # DVE binarize chain is the gate critical path).
            k = 0
            for side, j, order in (
                ("u", 0, (2, 0, 3, 1)),
                ("v", 0, (0, 2, 1, 3)),
                ("v", 1, (0, 2, 1, 3)),
            ):
                for c2 in order:
                    proj_tile(side, j, c2, pool=(psum_pool, ins_pool)[k % 2])
                    k += 1
            proj_tile("u", 1, 2, pool=ins_pool)
            proj_tile("u", 1, 0, pool=psum_pool)

            # Remaining projection tiles stream between main row-blocks
            # from the dedicated 2-slot PSUM pool, one DVE-fed plus one
            # scalar-fed tile per junction (parallel binarize, never two
            # Sign ops back to back). Every junction costs ~0.4us of PE
            # pipeline flush, so tiles ride in as few junctions as their
            # deadlines allow: u-chunk k before block 4k of the t=0 pass,
            # v2/v3 before the t=1 pass.
            inserts = {
                (0, 1): [("u", 1, 3), ("u", 1, 1)],
                (0, 3): [("u", 2, 2), ("u", 2, 0)],
                (0, 5): [("u", 2, 3), ("u", 2, 1)],
                (0, 7): [("u", 3, 2), ("u", 3, 0)],
                (0, 9): [("u", 3, 3), ("u", 3, 1)],
                (0, 11): [("v", 2, 0), ("v", 2, 2)],
                (0, 12): [("v", 2, 1), ("v", 2, 3)],
                (0, 13): [("v", 3, 0), ("v", 3, 2)],
                (0, 14): [("v", 3, 1), ("v", 3, 3)],
            }

            # Main code matmul in two column-half passes (fp8 DoubleRow,
            # integer-exact). The final block's Sin+store is split in half
            # so the last DMA is issued ~1us earlier.
            for t in range(2):
                for m in range(M1 // 128):
                    for ins in inserts.get((t, m), ()):
                        proj_tile(*ins, pool=ins_pool)
                    ms = slice(m * 128, (m + 1) * 128)
                    ot = out_pool.tile([128, 2 * NW], bf16)
                    ps = psum_pool.tile([128, 2, NW], f32, name="pstile", tag="ps")
                    for s in range(KC // 2):
                        for h in range(2):
                            n0 = t * 2 * NW + h * NW
                            nc.tensor.matmul(
                                ps[:, h, :],
                                ut[:, 2 * s : 2 * s + 2, ms],
                                vt[:, 2 * s : 2 * s + 2, n0 : n0 + NW],
                                start=(s == 0),
                                stop=(s == KC // 2 - 1),
                                perf_mode=mybir.MatmulPerfMode.DoubleRow,
                            )
                    last = t == 1 and m == M1 // 128 - 1
                    dma_eng = nc.sync if t == 0 else nc.gpsimd
                    if last:
                        for h in range(2):
                            nc.scalar.activation(
                                ot[:, h * NW : (h + 1) * NW],
                                ps[:, h, :],
                                mybir.ActivationFunctionType.Sin,
                                scale=scale,
                            )
                            (nc.sync if h == 0 else nc.gpsimd).dma_start(
                                out[ms, t * 2 * NW + h * NW : t * 2 * NW + (h + 1) * NW],
                                ot[:, h * NW : (h + 1) * NW],
                            )
                    else:
                        nc.scalar.activation(
                            ot[:],
                            ps[:],
                            mybir.ActivationFunctionType.Sin,
                            scale=scale,
                        )
                        dma_eng.dma_start(
                            out[ms, t * 2 * NW : (t + 1) * 2 * NW], ot[:]
                        )


    # Keep waits on the matmuls (not hoisted to ldweights) so redundant
    # weight loads stay sync-free and can be deduped away.
    nc.move_matmul_waits_to_ldweights = lambda: None
    nc.compile()
    _dedupe_ldweights(nc)
    _BUILD_CACHE[scale] = nc
    return nc


def _in_maps(emb1, emb2, r):
    r16 = np.ascontiguousarray(r.T).astype(np.float16)
    e1t = np.ascontiguousarray(emb1.T).astype(np.float16)
    e2t = np.ascontiguousarray(emb2.T).astype(np.float16)
    maps = []
    for k in range(8):
        a, b = k // G2, k % G2
        maps.append(
            {
                "e1": np.ascontiguousarray(e1t[:, a * M1 : (a + 1) * M1]),
                "e2": np.ascontiguousarray(e2t[:, b * M2 : (b + 1) * M2]),
                "r": r16,
            }
        )
    return maps


def _install_profile_hook():
    """The agent image's antenv lacks axon_hooks; synthesize it so
    run_bass_kernel_spmd(trace=True) can reach the NTFF profiler."""
    import types

    if "antenv.axon_hooks" in sys.modules:
        return
    try:
        from trn_agent_boot.trn_boot import _ntff_profile_via_ctypes

        hook = _ntff_profile_via_ctypes("/opt/axon/libaxon_pjrt.so")
        mod = types.ModuleType("antenv.axon_hooks")
        mod.get_axon_ntff_profile_hook = lambda: hook
        sys.modules["antenv.axon_hooks"] = mod

        from concourse import bass_utils as _bu

        _orig_upload = _bu.upload_artifacts

        def _safe_upload(tmpdir):
            try:
                return _orig_upload(tmpdir)
            except Exception as e:  # no bucket access in this container
                return f"upload-skipped: {e}"

        _bu.upload_artifacts = _safe_upload
    except Exception:
        pass


def kernel(emb1, emb2, r, pi, _trace=False, _tmpdir=None):
    emb1 = np.asarray(emb1, dtype=np.float32)
    emb2 = np.asarray(emb2, dtype=np.float32)
    r = np.asarray(r, dtype=np.float32)
    # codes multiply to (+/-1)*(+/-0.5): dot = s_u.s_v / 2, so the Sin
    # argument pi*s_u.s_v/2048 is dot * pi/1024
    scale = 2.0 * float(np.asarray(pi).reshape(-1)[0]) / (2.0 * B)

    nc = _build(scale)
    if _trace:
        _install_profile_hook()
    try:
        res = run_bass_kernel_spmd(
            nc, _in_maps(emb1, emb2, r), list(range(8)), trace=_trace, tmpdir=_tmpdir
        )
    except ModuleNotFoundError:
        res = run_bass_kernel_spmd(nc, _in_maps(emb1, emb2, r), list(range(8)))

    full = np.empty((N1, N2), dtype=np.float32)
    for k in range(8):
        a, b = k // G2, k % G2
        full[a * M1 : (a + 1) * M1, b * M2 : (b + 1) * M2] = np.asarray(
            res.results[k]["out"]
        ).astype(np.float32)
    if _trace:
        kernel._last_exec_time_ns = res.exec_time_ns
    return full


# revision 26
# speedup vs baseline: 1.1831x; 1.1831x over previous
"""LSH cosine-of-Hamming retrieval kernel for 8 trn2 NeuronCores.

Math: reference computes cos((pi/d) * hamming(u, v)) for binary LSH codes
u = (emb1 @ r.T > 0), v = (emb2 @ r.T > 0), d = 1024 bits.
With +/-1 sign codes s_u = 2u-1, s_v = 2v-1:
    hamming = (d - s_u . s_v) / 2
    cos((pi/d) * hamming) = sin((pi/2d) * s_u.s_v)

Projection runs as a single fp16 matmul pass (fp16 runs at bf16 rate on
the PE; the measured sign-flip rate ~9e-5 contributes ~1.3e-2 rel err,
inside the 2e-2 budget). Binarization of the projection PSUMs is split
across two engines: the DVE emits +/-0.5 codes (tensor_scalar is_gt/sub)
and the Activation engine emits +/-1 codes (Sign). Per 128-bit chunk the
two sides always use complementary conventions, so every bit contributes
(+/-1)*(+/-0.5) and the code dot is uniformly s_u.s_v/2 -> Sin scale
pi/1024. (GpSimd cannot help: it has no PSUM port.)

Main code matmul is fp8 DoubleRow (integer-exact in PSUM f32), split into
two column-half passes so the first half only needs v-chunks 0-1; the
u-side projection chunks 1-3 are interleaved between main row-block
groups, keeping the PE saturated end to end. Output is written bf16
(halves output DMA) and upcast on the host.

Sharding (2x4 grid over 8 cores): core k computes the [2048, 2048] output
block for emb1 rows [(k//4)*2048...] x emb2 rows [(k%4)*2048...]; r is
replicated (collectives cost ~60us fixed here - more than the projection
work they would dedupe).
"""

import sys

sys.path.insert(0, "/opt/trn_rl_repo")

import ml_dtypes
import numpy as np

import concourse.bacc as bacc
import concourse.tile as tile
from concourse import mybir
from concourse.bass_utils import run_bass_kernel_spmd

N1, N2, D, B = 4096, 8192, 128, 1024  # emb1 rows, emb2 rows, dim, num_bits
G1, G2 = 2, 4
M1, M2 = N1 // G1, N2 // G2  # 2048 x 2048 output block per core
KC = B // 128  # 8 bit-chunks of 128
RW = 512  # projection row-chunk width
NW = 512  # main matmul psum half-width

# bit-chunk PAIRS (of the KC//2 psum groups) whose u-side codes are made by
# the Activation engine as +/-1 (Sign); their v-side partners go to the DVE
# as +/-0.5. The complement set is the reverse. Keeps the convention product
# at 0.5 for every bit while splitting binarize work evenly per chunk.
SIGN_U_PAIRS = frozenset({0, 1})

_BUILD_CACHE = {}


def _dedupe_ldweights(nc):
    """Drop back-to-back InstLdweights with identical operands on the PE
    queue. The pipeline emits one weight load per matmul; when consecutive
    matmuls share a stationary operand, the reload is pure overhead. Only
    loads carrying no semaphore waits/updates are removed, so sync
    arithmetic is unchanged; the paired matmuls then use the weights the
    earlier identical load left in the array."""
    removed = 0
    for f in nc.m.functions:
        for bb in f.blocks:
            last_key = None
            for ins in list(bb.instructions):
                if type(ins).__name__ == "InstLdweights":
                    key = ins.concise()
                    if (
                        key == last_key
                        and not ins.has_wait()
                        and not ins.has_update()
                    ):
                        bb.instructions.remove(ins)
                        removed += 1
                    else:
                        last_key = key
    return removed


def _build(scale: float):
    if scale in _BUILD_CACHE:
        return _BUILD_CACHE[scale]
    nc = bacc.Bacc("TRN2", target_bir_lowering=False, debug=False)
    f32 = mybir.dt.float32
    f16 = mybir.dt.float16
    bf16 = mybir.dt.bfloat16
    fp8 = mybir.dt.float8e4

    e1 = nc.declare_dram_parameter("e1", [D, M1], f16, isOutput=False)
    e2 = nc.declare_dram_parameter("e2", [D, M2], f16, isOutput=False)
    r = nc.declare_dram_parameter("r", [D, B], f16, isOutput=False)
    out = nc.declare_dram_parameter("out", [M1, M2], bf16, isOutput=True)

    with tile.TileContext(nc) as tc:
        with (
            tc.tile_pool(name="const", bufs=1) as const_pool,
            tc.tile_pool(name="codes", bufs=1) as code_pool,
            tc.tile_pool(name="outs", bufs=3) as out_pool,
            tc.tile_pool(name="psum", bufs=2, space="PSUM") as psum_pool,
            tc.tile_pool(name="pjp", bufs=2, space="PSUM") as ins_pool,
        ):
            r_sb = const_pool.tile([D, B], f16)
            e1_sb = const_pool.tile([D, M1], f16)
            e2_sb = const_pool.tile([D, M2], f16)
            ut = code_pool.tile([128, KC, M1], fp8)
            vt = code_pool.tile([128, KC, M2], fp8)

            # Input DMAs up front, in consumption order. sync queue carries
            # the gating pieces (first r bit-chunk, first e1 row-chunk);
            # gpsimd carries the e2 chunks. Output DMAs come later on both.
            # The DMA fabric drains each queue roughly in order at ~1.5us
            # latency + shared bandwidth, so the ~1.3MB input set is staged:
            # the prologue-gating transfers (r pieces in u0's bit-pair
            # order, e1 chunk 0, e2 chunk 0) lead the two queues, and the
            # four chunks not needed until the main phase ride the tails.
            def edma(q, esb, edr, j):
                sl = slice(j * RW, (j + 1) * RW)
                q.dma_start(esb[:, sl], edr[:, sl])

            nc.gpsimd.dma_start(e1_sb[:, 0:RW], e1[:, 0:RW])
            nc.sync.dma_start(r_sb[:, 512:768], r[:, 512:768])
            edma(nc.gpsimd, e2_sb, e2, 0)
            nc.sync.dma_start(r_sb[:, 0:512], r[:, 0:512])
            nc.sync.dma_start(r_sb[:, 768:], r[:, 768:])
            edma(nc.sync, e2_sb, e2, 1)
            edma(nc.sync, e1_sb, e1, 1)
            edma(nc.gpsimd, e2_sb, e2, 2)
            edma(nc.gpsimd, e2_sb, e2, 3)
            edma(nc.gpsimd, e1_sb, e1, 2)
            edma(nc.sync, e1_sb, e1, 3)

            # Narrow HAM warm-up matmuls bridge the input-DMA wait: an idle
            # gap before the first projection matmul would reset the PE
            # clock ramp (~3.4us of SUSTAINED activity needed for 2.4 GHz),
            # and 256-wide dummies waste at most ~0.2us once data lands.
            # The dummy Sin (into a scratch tile, so the warm-ups don't
            # depend on it) makes the activation table pass load the trig
            # table - which serves BOTH Sin and Sign - once, instead of
            # reloading at the first mid-stream function switch.
            warm = const_pool.tile([128, RW], bf16)
            scratch = const_pool.tile([128, 1], bf16)
            nc.vector.memset(warm[:], 0.0)
            nc.scalar.activation(
                scratch[:], warm[:, 0:1],
                mybir.ActivationFunctionType.Sin, scale=scale,
            )
            wps = psum_pool.tile([128, 2, RW], f32, name="pstile", tag="ps")
            for w in range(14):
                nc.tensor.matmul(
                    wps[:, w % 2, 0:256], warm[:, 0:128], warm[:, 0:256],
                    start=True, stop=True,
                )

            def proj_tile(side, j, c2, pool=psum_pool):
                esb = e1_sb if side == "u" else e2_sb
                dst = ut if side == "u" else vt
                sl = slice(j * RW, (j + 1) * RW)
                tg = "ps" if pool is psum_pool else "pj"
                ps = pool.tile([128, 2, RW], f32, name="pstile", tag=tg)
                for h in range(2):
                    cs = slice((2 * c2 + h) * 128, (2 * c2 + h + 1) * 128)
                    nc.tensor.matmul(
                        ps[:, h, :], r_sb[:, cs], esb[:, sl],
                        start=True, stop=True,
                    )
                code = dst[:, 2 * c2 : 2 * c2 + 2, sl]
                use_sign = (c2 in SIGN_U_PAIRS) == (side == "u")
                if use_sign:
                    nc.scalar.activation(
                        code, ps[:], mybir.ActivationFunctionType.Sign
                    )
                else:
                    nc.vector.tensor_scalar(
                        code,
                        ps[:],
                        0.0,
                        0.5,
                        mybir.AluOpType.is_gt,
                        mybir.AluOpType.subtract,
                    )

            # Strict prologue: only the tiles the first main block needs
            # (u rows 0-511, v columns 0-1023), each chunk's bit-pairs
            # alternating between the two binarize engines so neither
            # starves, and the tiles alternating between the two PSUM pools
            # so the effective rotation is 4 deep (a 2-deep rotation makes
            # the PE wait a binarize + two semaphore hops per tile). The u1
            # pre-inserts fill the tensor idle at the end of the gate (the
            # DVE binarize chain is the gate critical path).
            k = 0
            for side, j, order in (
                ("u", 0, (2, 0, 3, 1)),
                ("v", 0, (0, 2, 1, 3)),
                ("v", 1, (0, 2, 1, 3)),
            ):
                for c2 in order:
                    proj_tile(side, j, c2, pool=(psum_pool, ins_pool)[k % 2])
                    k += 1
            proj_tile("u", 1, 2, pool=ins_pool)
            proj_tile("u", 1, 0, pool=psum_pool)

            # Remaining projection tiles stream between main row-blocks
            # from the dedicated 2-slot PSUM pool, one DVE-fed plus one
            # scalar-fed tile per junction (parallel binarize, never two
            # Sign ops back to back). Every junction costs ~0.4us of PE
            # pipeline flush, so tiles ride in as few junctions as their
            # deadlines allow: u-chunk k before block 4k of the t=0 pass,
            # v2/v3 before the t=1 pass.
            inserts = {
                (0, 1): [("u", 1, 3), ("u", 1, 1)],
                (0, 3): [("u", 2, 2), ("u", 2, 0)],
                (0, 5): [("u", 2, 3), ("u", 2, 1)],
                (0, 7): [("u", 3, 2), ("u", 3, 0)],
                (0, 9): [("u", 3, 3), ("u", 3, 1)],
                (0, 11): [("v", 2, 0), ("v", 2, 2)],
                (0, 12): [("v", 2, 1), ("v", 2, 3)],
                (0, 13): [("v", 3, 0), ("v", 3, 2)],
                (0, 14): [("v", 3, 1), ("v", 3, 3)],
            }

            # Main code matmul in two column-half passes (fp8 DoubleRow,
            # integer-exact). The final block's Sin+store is split in half
            # so the last DMA is issued ~1us earlier.
            for t in range(2):
                for m in range(M1 // 128):
                    for ins in inserts.get((t, m), ()):
                        proj_tile(*ins, pool=ins_pool)
                    ms = slice(m * 128, (m + 1) * 128)
                    ot = out_pool.tile([128, 2 * NW], bf16)
                    ps = psum_pool.tile([128, 2, NW], f32, name="pstile", tag="ps")
                    last = t == 1 and m == M1 // 128 - 1
                    if last:
                        # h-major matmul order for the final block: its
                        # first half-Sin and output DMA overlap the second
                        # half's matmuls, trimming the kernel tail.
                        for h in range(2):
                            for s in range(KC // 2):
                                n0 = t * 2 * NW + h * NW
                                nc.tensor.matmul(
                                    ps[:, h, :],
                                    ut[:, 2 * s : 2 * s + 2, ms],
                                    vt[:, 2 * s : 2 * s + 2, n0 : n0 + NW],
                                    start=(s == 0),
                                    stop=(s == KC // 2 - 1),
                                    perf_mode=mybir.MatmulPerfMode.DoubleRow,
                                )
                            nc.scalar.activation(
                                ot[:, h * NW : (h + 1) * NW],
                                ps[:, h, :],
                                mybir.ActivationFunctionType.Sin,
                                scale=scale,
                            )
                            (nc.sync if h == 0 else nc.gpsimd).dma_start(
                                out[ms, t * 2 * NW + h * NW : t * 2 * NW + (h + 1) * NW],
                                ot[:, h * NW : (h + 1) * NW],
                            )
                        continue
                    for s in range(KC // 2):
                        for h in range(2):
                            n0 = t * 2 * NW + h * NW
                            nc.tensor.matmul(
                                ps[:, h, :],
                                ut[:, 2 * s : 2 * s + 2, ms],
                                vt[:, 2 * s : 2 * s + 2, n0 : n0 + NW],
                                start=(s == 0),
                                stop=(s == KC // 2 - 1),
                                perf_mode=mybir.MatmulPerfMode.DoubleRow,
                            )
                    nc.scalar.activation(
                        ot[:],
                        ps[:],
                        mybir.ActivationFunctionType.Sin,
                        scale=scale,
                    )
                    (nc.sync if t == 0 else nc.gpsimd).dma_start(
                        out[ms, t * 2 * NW : (t + 1) * 2 * NW], ot[:]
                    )


    # Keep waits on the matmuls (not hoisted to ldweights) so redundant
    # weight loads stay sync-free and can be deduped away.
    nc.move_matmul_waits_to_ldweights = lambda: None
    nc.compile()
    _dedupe_ldweights(nc)
    _BUILD_CACHE[scale] = nc
    return nc


def _in_maps(emb1, emb2, r):
    r16 = np.ascontiguousarray(r.T).astype(np.float16)
    e1t = np.ascontiguousarray(emb1.T).astype(np.float16)
    e2t = np.ascontiguousarray(emb2.T).astype(np.float16)
    maps = []
    for k in range(8):
        a, b = k // G2, k % G2
        maps.append(
            {
                "e1": np.ascontiguousarray(e1t[:, a * M1 : (a + 1) * M1]),
                "e2": np.ascontiguousarray(e2t[:, b * M2 : (b + 1) * M2]),
                "r": r16,
            }
        )
    return maps


def _install_profile_hook():
    """The agent image's antenv lacks axon_hooks; synthesize it so
    run_bass_kernel_spmd(trace=True) can reach the NTFF profiler."""
    import types

    if "antenv.axon_hooks" in sys.modules:
        return
    try:
        from trn_agent_boot.trn_boot import _ntff_profile_via_ctypes

        hook = _ntff_profile_via_ctypes("/opt/axon/libaxon_pjrt.so")
        mod = types.ModuleType("antenv.axon_hooks")
        mod.get_axon_ntff_profile_hook = lambda: hook
        sys.modules["antenv.axon_hooks"] = mod

        from concourse import bass_utils as _bu

        _orig_upload = _bu.upload_artifacts

        def _safe_upload(tmpdir):
            try:
                return _orig_upload(tmpdir)
            except Exception as e:  # no bucket access in this container
                return f"upload-skipped: {e}"

        _bu.upload_artifacts = _safe_upload
    except Exception:
        pass


def kernel(emb1, emb2, r, pi, _trace=False, _tmpdir=None):
    emb1 = np.asarray(emb1, dtype=np.float32)
    emb2 = np.asarray(emb2, dtype=np.float32)
    r = np.asarray(r, dtype=np.float32)
    # codes multiply to (+/-1)*(+/-0.5): dot = s_u.s_v / 2, so the Sin
    # argument pi*s_u.s_v/2048 is dot * pi/1024
    scale = 2.0 * float(np.asarray(pi).reshape(-1)[0]) / (2.0 * B)

    nc = _build(scale)
    if _trace:
        _install_profile_hook()
    try:
        res = run_bass_kernel_spmd(
            nc, _in_maps(emb1, emb2, r), list(range(8)), trace=_trace, tmpdir=_tmpdir
        )
    except ModuleNotFoundError:
        res = run_bass_kernel_spmd(nc, _in_maps(emb1, emb2, r), list(range(8)))

    full = np.empty((N1, N2), dtype=np.float32)
    for k in range(8):
        a, b = k // G2, k % G2
        full[a * M1 : (a + 1) * M1, b * M2 : (b + 1) * M2] = np.asarray(
            res.results[k]["out"]
        ).astype(np.float32)
    if _trace:
        kernel._last_exec_time_ns = res.exec_time_ns
    return full


# revision 29
# speedup vs baseline: 1.1877x; 1.0039x over previous
"""LSH cosine-of-Hamming retrieval kernel for 8 trn2 NeuronCores.

Math: reference computes cos((pi/d) * hamming(u, v)) for binary LSH codes
u = (emb1 @ r.T > 0), v = (emb2 @ r.T > 0), d = 1024 bits.
With +/-1 sign codes s_u = 2u-1, s_v = 2v-1:
    hamming = (d - s_u . s_v) / 2
    cos((pi/d) * hamming) = sin((pi/2d) * s_u.s_v)

Projection runs as a single fp16 matmul pass (fp16 runs at bf16 rate on
the PE; the measured sign-flip rate ~9e-5 contributes ~1.3e-2 rel err,
inside the 2e-2 budget). Binarization of the projection PSUMs is split
across two engines: the DVE emits +/-0.5 codes (tensor_scalar is_gt/sub)
and the Activation engine emits +/-1 codes (Sign). Per 128-bit chunk the
two sides always use complementary conventions, so every bit contributes
(+/-1)*(+/-0.5) and the code dot is uniformly s_u.s_v/2 -> Sin scale
pi/1024. (GpSimd cannot help: it has no PSUM port.)

Main code matmul is fp8 DoubleRow (integer-exact in PSUM f32), split into
two column-half passes so the first half only needs v-chunks 0-1; the
u-side projection chunks 1-3 are interleaved between main row-block
groups, keeping the PE saturated end to end. Output is written bf16
(halves output DMA) and upcast on the host.

Sharding (2x4 grid over 8 cores): core k computes the [2048, 2048] output
block for emb1 rows [(k//4)*2048...] x emb2 rows [(k%4)*2048...]; r is
replicated (collectives cost ~60us fixed here - more than the projection
work they would dedupe).
"""

import sys

sys.path.insert(0, "/opt/trn_rl_repo")

import ml_dtypes
import numpy as np

import concourse.bacc as bacc
import concourse.tile as tile
from concourse import mybir
from concourse.bass_utils import run_bass_kernel_spmd

N1, N2, D, B = 4096, 8192, 128, 1024  # emb1 rows, emb2 rows, dim, num_bits
G1, G2 = 2, 4
M1, M2 = N1 // G1, N2 // G2  # 2048 x 2048 output block per core
KC = B // 128  # 8 bit-chunks of 128
RW = 512  # projection row-chunk width
NW = 512  # main matmul psum half-width

# bit-chunk PAIRS (of the KC//2 psum groups) whose u-side codes are made by
# the Activation engine as +/-1 (Sign); their v-side partners go to the DVE
# as +/-0.5. The complement set is the reverse. Keeps the convention product
# at 0.5 for every bit while splitting binarize work evenly per chunk.
SIGN_U_PAIRS = frozenset({0, 1})

_BUILD_CACHE = {}


def _dedupe_ldweights(nc):
    """Drop back-to-back InstLdweights with identical operands on the PE
    queue. The pipeline emits one weight load per matmul; when consecutive
    matmuls share a stationary operand, the reload is pure overhead. Only
    loads carrying no semaphore waits/updates are removed, so sync
    arithmetic is unchanged; the paired matmuls then use the weights the
    earlier identical load left in the array."""
    removed = 0
    for f in nc.m.functions:
        for bb in f.blocks:
            last_key = None
            for ins in list(bb.instructions):
                if type(ins).__name__ == "InstLdweights":
                    key = ins.concise()
                    if (
                        key == last_key
                        and not ins.has_wait()
                        and not ins.has_update()
                    ):
                        bb.instructions.remove(ins)
                        removed += 1
                    else:
                        last_key = key
    return removed


def _build(scale: float):
    if scale in _BUILD_CACHE:
        return _BUILD_CACHE[scale]
    nc = bacc.Bacc("TRN2", target_bir_lowering=False, debug=False)
    f32 = mybir.dt.float32
    f16 = mybir.dt.float16
    bf16 = mybir.dt.bfloat16
    fp8 = mybir.dt.float8e4

    e1 = nc.declare_dram_parameter("e1", [D, M1], f16, isOutput=False)
    e2 = nc.declare_dram_parameter("e2", [D, M2], f16, isOutput=False)
    r = nc.declare_dram_parameter("r", [D, B], f16, isOutput=False)
    out = nc.declare_dram_parameter("out", [M1, M2], bf16, isOutput=True)

    with tile.TileContext(nc) as tc:
        with (
            tc.tile_pool(name="const", bufs=1) as const_pool,
            tc.tile_pool(name="codes", bufs=1) as code_pool,
            tc.tile_pool(name="outs", bufs=3) as out_pool,
            tc.tile_pool(name="psum", bufs=2, space="PSUM") as psum_pool,
            tc.tile_pool(name="pjp", bufs=2, space="PSUM") as ins_pool,
        ):
            r_sb = const_pool.tile([D, B], f16)
            e1_sb = const_pool.tile([D, M1], f16)
            e2_sb = const_pool.tile([D, M2], f16)
            ut = code_pool.tile([128, KC, M1], fp8)
            vt = code_pool.tile([128, KC, M2], fp8)

            # Input DMAs up front, in consumption order. sync queue carries
            # the gating pieces (first r bit-chunk, first e1 row-chunk);
            # gpsimd carries the e2 chunks. Output DMAs come later on both.
            # The DMA fabric drains each queue roughly in order at ~1.5us
            # latency + shared bandwidth, so the ~1.3MB input set is staged:
            # the prologue-gating transfers (r pieces in u0's bit-pair
            # order, e1 chunk 0, e2 chunk 0) lead the two queues, and the
            # four chunks not needed until the main phase ride the tails.
            def edma(q, esb, edr, j):
                sl = slice(j * RW, (j + 1) * RW)
                q.dma_start(esb[:, sl], edr[:, sl])

            nc.gpsimd.dma_start(e1_sb[:, 0:RW], e1[:, 0:RW])
            nc.sync.dma_start(r_sb[:, 512:768], r[:, 512:768])
            edma(nc.gpsimd, e2_sb, e2, 0)
            nc.sync.dma_start(r_sb[:, 0:512], r[:, 0:512])
            nc.sync.dma_start(r_sb[:, 768:], r[:, 768:])
            edma(nc.sync, e2_sb, e2, 1)
            edma(nc.sync, e1_sb, e1, 1)
            edma(nc.gpsimd, e2_sb, e2, 2)
            edma(nc.gpsimd, e2_sb, e2, 3)
            edma(nc.gpsimd, e1_sb, e1, 2)
            edma(nc.sync, e1_sb, e1, 3)

            # Narrow HAM warm-up matmuls bridge the input-DMA wait: an idle
            # gap before the first projection matmul would reset the PE
            # clock ramp (~3.4us of SUSTAINED activity needed for 2.4 GHz),
            # and 256-wide dummies waste at most ~0.2us once data lands.
            # The dummy Sin (into a scratch tile, so the warm-ups don't
            # depend on it) makes the activation table pass load the trig
            # table - which serves BOTH Sin and Sign - once, instead of
            # reloading at the first mid-stream function switch.
            warm = const_pool.tile([128, RW], bf16)
            scratch = const_pool.tile([128, 1], bf16)
            nc.vector.memset(warm[:], 0.0)
            nc.scalar.activation(
                scratch[:], warm[:, 0:1],
                mybir.ActivationFunctionType.Sin, scale=scale,
            )
            wps = psum_pool.tile([128, 2, RW], f32, name="pstile", tag="ps")
            for w in range(14):
                nc.tensor.matmul(
                    wps[:, w % 2, 0:256], warm[:, 0:128], warm[:, 0:256],
                    start=True, stop=True,
                )

            def proj_tile(side, j, c2, pool=psum_pool):
                esb = e1_sb if side == "u" else e2_sb
                dst = ut if side == "u" else vt
                sl = slice(j * RW, (j + 1) * RW)
                tg = "ps" if pool is psum_pool else "pj"
                ps = pool.tile([128, 2, RW], f32, name="pstile", tag=tg)
                for h in range(2):
                    cs = slice((2 * c2 + h) * 128, (2 * c2 + h + 1) * 128)
                    nc.tensor.matmul(
                        ps[:, h, :], r_sb[:, cs], esb[:, sl],
                        start=True, stop=True,
                    )
                code = dst[:, 2 * c2 : 2 * c2 + 2, sl]
                use_sign = (c2 in SIGN_U_PAIRS) == (side == "u")
                if use_sign:
                    nc.scalar.activation(
                        code, ps[:], mybir.ActivationFunctionType.Sign
                    )
                else:
                    nc.vector.tensor_scalar(
                        code,
                        ps[:],
                        0.0,
                        0.5,
                        mybir.AluOpType.is_gt,
                        mybir.AluOpType.subtract,
                    )

            # Strict prologue, explicitly scheduled. Bit-pairs alternate
            # between the two binarize engines (neither starves) and tiles
            # alternate between the two PSUM pools (effective 4-deep
            # rotation; 2-deep makes the PE wait a binarize + two semaphore
            # hops per tile). The four tiles the main pool keeps (u0/v0,
            # early in both binarize chains) are exactly the slots blocks
            # b0/b1 will reuse, so those blocks are not gated on the v1
            # chain tail.
            for side, j, c2, pool in (
                ("u", 0, 2, psum_pool), ("u", 0, 3, ins_pool),
                ("u", 0, 0, psum_pool), ("u", 0, 1, ins_pool),
                ("v", 0, 0, psum_pool), ("v", 0, 1, ins_pool),
                ("v", 0, 2, psum_pool), ("v", 0, 3, ins_pool),
                ("v", 1, 0, ins_pool), ("v", 1, 2, ins_pool),
            ):
                proj_tile(side, j, c2, pool=pool)

            # Blocks 0 and 1 of the t=0 pass run h-split: their h=0 halves
            # need only the u0/v0 codes, so they start ~3us before the v1
            # binarize chain finishes; the remaining v1 tiles ride between
            # the half-passes and the u1 pre-inserts after them.
            def dr_group(ps, h, t, m):
                ms = slice(m * 128, (m + 1) * 128)
                for s in range(KC // 2):
                    n0 = t * 2 * NW + h * NW
                    nc.tensor.matmul(
                        ps[:, h, :],
                        ut[:, 2 * s : 2 * s + 2, ms],
                        vt[:, 2 * s : 2 * s + 2, n0 : n0 + NW],
                        start=(s == 0),
                        stop=(s == KC // 2 - 1),
                        perf_mode=mybir.MatmulPerfMode.DoubleRow,
                    )

            early = []
            for m in range(2):
                ps = psum_pool.tile([128, 2, NW], f32, name="pstile", tag="ps")
                early.append(ps)
                dr_group(ps, 0, 0, m)
            proj_tile("v", 1, 1, pool=ins_pool)
            proj_tile("v", 1, 3, pool=ins_pool)
            for m in range(2):
                dr_group(early[m], 1, 0, m)
                ot = out_pool.tile([128, 2 * NW], bf16)
                nc.scalar.activation(
                    ot[:], early[m][:],
                    mybir.ActivationFunctionType.Sin, scale=scale,
                )
                nc.sync.dma_start(out[m * 128 : (m + 1) * 128, 0 : 2 * NW], ot[:])
            proj_tile("u", 1, 2, pool=ins_pool)
            proj_tile("u", 1, 0, pool=ins_pool)

            # Remaining projection tiles stream between main row-blocks
            # from the dedicated 2-slot PSUM pool, one DVE-fed plus one
            # scalar-fed tile per junction (parallel binarize, never two
            # Sign ops back to back). Every junction costs ~0.4us of PE
            # pipeline flush, so tiles ride in as few junctions as their
            # deadlines allow: u-chunk k before block 4k of the t=0 pass,
            # v2/v3 before the t=1 pass.
            inserts = {
                (0, 2): [("u", 1, 3), ("u", 1, 1)],
                (0, 3): [("u", 2, 2), ("u", 2, 0)],
                (0, 5): [("u", 2, 3), ("u", 2, 1)],
                (0, 7): [("u", 3, 2), ("u", 3, 0)],
                (0, 9): [("u", 3, 3), ("u", 3, 1)],
                (0, 11): [("v", 2, 0), ("v", 2, 2)],
                (0, 12): [("v", 2, 1), ("v", 2, 3)],
                (0, 13): [("v", 3, 0), ("v", 3, 2)],
                (0, 14): [("v", 3, 1), ("v", 3, 3)],
            }

            # Main code matmul in two column-half passes (fp8 DoubleRow,
            # integer-exact). The final block's Sin+store is split in half
            # so the last DMA is issued ~1us earlier.
            for t in range(2):
                for m in range(2 if t == 0 else 0, M1 // 128):
                    for ins in inserts.get((t, m), ()):
                        proj_tile(*ins, pool=ins_pool)
                    ms = slice(m * 128, (m + 1) * 128)
                    ot = out_pool.tile([128, 2 * NW], bf16)
                    ps = psum_pool.tile([128, 2, NW], f32, name="pstile", tag="ps")
                    last = t == 1 and m == M1 // 128 - 1
                    if last:
                        # h-major matmul order for the final block, with the
                        # second half in its own PSUM tile (sharing one tile
                        # makes the h=1 matmuls wait on the h=0 Sin): the
                        # first half-Sin and output DMA overlap the second
                        # half's matmuls, trimming the kernel tail.
                        ps2 = ins_pool.tile([128, 2, NW], f32, name="pstile", tag="pj")
                        for h, pst in ((0, ps), (1, ps2)):
                            dr_group(pst, h, t, m)
                            nc.scalar.activation(
                                ot[:, h * NW : (h + 1) * NW],
                                pst[:, h, :],
                                mybir.ActivationFunctionType.Sin,
                                scale=scale,
                            )
                            (nc.sync if h == 0 else nc.gpsimd).dma_start(
                                out[ms, t * 2 * NW + h * NW : t * 2 * NW + (h + 1) * NW],
                                ot[:, h * NW : (h + 1) * NW],
                            )
                        continue
                    for s in range(KC // 2):
                        for h in range(2):
                            n0 = t * 2 * NW + h * NW
                            nc.tensor.matmul(
                                ps[:, h, :],
                                ut[:, 2 * s : 2 * s + 2, ms],
                                vt[:, 2 * s : 2 * s + 2, n0 : n0 + NW],
                                start=(s == 0),
                                stop=(s == KC // 2 - 1),
                                perf_mode=mybir.MatmulPerfMode.DoubleRow,
                            )
                    nc.scalar.activation(
                        ot[:],
                        ps[:],
                        mybir.ActivationFunctionType.Sin,
                        scale=scale,
                    )
                    (nc.sync if t == 0 else nc.gpsimd).dma_start(
                        out[ms, t * 2 * NW : (t + 1) * 2 * NW], ot[:]
                    )


    # Keep waits on the matmuls (not hoisted to ldweights) so redundant
    # weight loads stay sync-free and can be deduped away.
    nc.move_matmul_waits_to_ldweights = lambda: None
    nc.compile()
    _dedupe_ldweights(nc)
    _BUILD_CACHE[scale] = nc
    return nc


def _in_maps(emb1, emb2, r):
    r16 = np.ascontiguousarray(r.T).astype(np.float16)
    e1t = np.ascontiguousarray(emb1.T).astype(np.float16)
    e2t = np.ascontiguousarray(emb2.T).astype(np.float16)
    maps = []
    for k in range(8):
        a, b = k // G2, k % G2
        maps.append(
            {
                "e1": np.ascontiguousarray(e1t[:, a * M1 : (a + 1) * M1]),
                "e2": np.ascontiguousarray(e2t[:, b * M2 : (b + 1) * M2]),
                "r": r16,
            }
        )
    return maps


def _install_profile_hook():
    """The agent image's antenv lacks axon_hooks; synthesize it so
    run_bass_kernel_spmd(trace=True) can reach the NTFF profiler."""
    import types

    if "antenv.axon_hooks" in sys.modules:
        return
    try:
        from trn_agent_boot.trn_boot import _ntff_profile_via_ctypes

        hook = _ntff_profile_via_ctypes("/opt/axon/libaxon_pjrt.so")
        mod = types.ModuleType("antenv.axon_hooks")
        mod.get_axon_ntff_profile_hook = lambda: hook
        sys.modules["antenv.axon_hooks"] = mod

        from concourse import bass_utils as _bu

        _orig_upload = _bu.upload_artifacts

        def _safe_upload(tmpdir):
            try:
                return _orig_upload(tmpdir)
            except Exception as e:  # no bucket access in this container
                return f"upload-skipped: {e}"

        _bu.upload_artifacts = _safe_upload
    except Exception:
        pass


def kernel(emb1, emb2, r, pi, _trace=False, _tmpdir=None):
    emb1 = np.asarray(emb1, dtype=np.float32)
    emb2 = np.asarray(emb2, dtype=np.float32)
    r = np.asarray(r, dtype=np.float32)
    # codes multiply to (+/-1)*(+/-0.5): dot = s_u.s_v / 2, so the Sin
    # argument pi*s_u.s_v/2048 is dot * pi/1024
    scale = 2.0 * float(np.asarray(pi).reshape(-1)[0]) / (2.0 * B)

    nc = _build(scale)
    if _trace:
        _install_profile_hook()
    try:
        res = run_bass_kernel_spmd(
            nc, _in_maps(emb1, emb2, r), list(range(8)), trace=_trace, tmpdir=_tmpdir
        )
    except ModuleNotFoundError:
        res = run_bass_kernel_spmd(nc, _in_maps(emb1, emb2, r), list(range(8)))

    full = np.empty((N1, N2), dtype=np.float32)
    for k in range(8):
        a, b = k // G2, k % G2
        full[a * M1 : (a + 1) * M1, b * M2 : (b + 1) * M2] = np.asarray(
            res.results[k]["out"]
        ).astype(np.float32)
    if _trace:
        kernel._last_exec_time_ns = res.exec_time_ns
    return full
